# revision 2
# baseline (speedup 1.0000x reference)
"""Trainium2 Bass kernel for AttentionWithSharedWeights — interleaved v4.

Sharding: batch (4) x head-group (2) over 8 cores; host sums the two output
partials per batch.

Key ideas vs baseline:
  - Phase B's softmax exp stream (Activation engine) is the serialized
    bottleneck; phases A (QKV proj) and C (output proj) are PE-bound.  All
    three are emitted INTERLEAVED so exp hides under projection matmuls.
  - Moving matmul operands in f32r (0.75 cycles/col on HW), stationary
    operands in bf16 (halves SBUF) when MIXED=True.
  - No ot spill: B and C fused per q-chunk via SBUF o tiles.
  - y written bf16 (and transposed: y_d is [DIM, S]); Q spilled bf16.
  - Diagonal r=3 tile widened to 256 cols to dodge the f32r <256-col
    penalty; a combined zero+triangle mask fixes it up.
"""

import numpy as np
import ml_dtypes

B, S, DIM = 4, 2048, 2048
NH, NKV, HD = 16, 4, 128
SC = 512
NSC = S // SC    # 4
NKT = S // HD    # 16
NKH = 4          # x chunk quarters (k-tiles per x tile)
HPC = NH // 2    # 8
KVPC = NKV // 2  # 2
FQ = HPC * HD    # 1024
FKV = KVPC * HD  # 256
SCALE = 1.0 / float(np.sqrt(HD))
MIXED = False    # f32r moving operands vs all-bf16

_cache = {}


def _build_program():
    import concourse.mybir as mybir
    import concourse.tile as tile
    from concourse import bacc

    f32 = mybir.dt.float32
    f32r = mybir.dt.float32r
    bf16 = mybir.dt.bfloat16
    mv = f32r if MIXED else bf16      # moving-operand dtype
    mvnp = f32 if MIXED else bf16     # same, for dram params
    Exp = mybir.ActivationFunctionType.Exp

    nc = bacc.Bacc()

    xt_d = nc.declare_dram_parameter("xt", [DIM, S], mvnp, isOutput=False)
    wq_d = nc.declare_dram_parameter("wq", [DIM, FQ], bf16, isOutput=False)
    wk_d = nc.declare_dram_parameter("wk", [DIM, FKV], bf16, isOutput=False)
    wv_d = nc.declare_dram_parameter("wv", [DIM, FKV], bf16, isOutput=False)
    wo_d = nc.declare_dram_parameter("wo", [FQ, DIM], bf16, isOutput=False)
    cs_d = nc.declare_dram_parameter("cs", [HD, S], bf16, isOutput=False)
    sn_d = nc.declare_dram_parameter("sn", [HD, S], bf16, isOutput=False)
    rt_d = nc.declare_dram_parameter("rt", [HD, HD], bf16, isOutput=False)
    on_d = nc.declare_dram_parameter("on", [HD, HD], mvnp, isOutput=False)
    mk_d = nc.declare_dram_parameter("mk", [HD, 2, HD], f32, isOutput=False)
    mk3_d = nc.declare_dram_parameter("mk3", [HD, 2 * HD], f32, isOutput=False)
    y_d = nc.declare_dram_parameter("y", [DIM, S], bf16, isOutput=True)

    qt_ds = [nc.dram_tensor(f"qt_spill{i}", [FQ, SC], bf16) for i in range(NSC)]

    def _pull(g):
        try:
            next(g)
            return True
        except StopIteration:
            return False

    with tile.TileContext(nc) as tc:
        kt_t = nc.alloc_sbuf_tensor("kt_res", [HD, KVPC, S], bf16)
        v_t = nc.alloc_sbuf_tensor("v_res", [HD, NKT, FKV], bf16)
        kt_sb = kt_t[:]
        v_sb = v_t[:]

        with tc.tile_pool(name="pw", bufs=1) as pw, \
             tc.tile_pool(name="px", bufs=4) as px, \
             tc.tile_pool(name="pt", bufs=2) as pt, \
             tc.tile_pool(name="pr", bufs=2) as pr, \
             tc.tile_pool(name="pq", bufs=3) as pq, \
             tc.tile_pool(name="pe", bufs=4) as pe, \
             tc.tile_pool(name="po", bufs=2) as po, \
             tc.tile_pool(name="pn", bufs=3) as pn, \
             tc.tile_pool(name="py", bufs=4) as py, \
             tc.tile_pool(name="ps_q", bufs=1, space="PSUM") as ps_q, \
             tc.tile_pool(name="ps_r", bufs=1, space="PSUM") as ps_r, \
             tc.tile_pool(name="ps_v", bufs=1, space="PSUM") as ps_v, \
             tc.tile_pool(name="ps_s", bufs=1, space="PSUM") as ps_s, \
             tc.tile_pool(name="ps_o", bufs=1, space="PSUM") as ps_o, \
             tc.tile_pool(name="ps_b", bufs=1, space="PSUM") as ps_b, \
             tc.tile_pool(name="ps_y", bufs=1, space="PSUM") as ps_y:

            # ---- resident weights / tables ----
            wq_sb = pw.tile([HD, NKT, FQ], bf16)
            wk_sb = pw.tile([HD, NKT, FKV], bf16)
            wv_sb = pw.tile([HD, NKT, FKV], bf16)
            wo_sb = pw.tile([HD, HPC, DIM], bf16)
            rt_sb = pw.tile([HD, HD], bf16)
            on_sb = pw.tile([HD, HD], mv)
            mk_sb = pw.tile([HD, 2, HD], f32)
            mk3_sb = pw.tile([HD, 2 * HD], f32)
            for ft in range(HPC):
                nc.sync.dma_start(
                    wq_sb[:, :, ft * HD:(ft + 1) * HD],
                    wq_d[:, ft * HD:(ft + 1) * HD].rearrange(
                        "(k p) f -> p k f", p=HD))
            nc.sync.dma_start(wk_sb[:], wk_d[:].rearrange("(k p) f -> p k f", p=HD))
            nc.sync.dma_start(wv_sb[:], wv_d[:].rearrange("(k p) f -> p k f", p=HD))
            nc.sync.dma_start(rt_sb[:], rt_d[:])
            nc.sync.dma_start(on_sb[:], on_d[:])
            nc.sync.dma_start(mk_sb[:], mk_d[:])
            nc.sync.dma_start(mk3_sb[:], mk3_d[:])
            for ft in range(HPC):
                nc.sync.dma_start(
                    wo_sb[:, ft, :], wo_d[ft * HD:(ft + 1) * HD, :])

            # ---- phase A generator: one chunk = 11+ yielded units ----
            def phase_a(sc):
                ssl = slice(sc * SC, (sc + 1) * SC)
                xcs = []
                for half in range(NKT // NKH):
                    xc = px.tile([HD, NKH, SC], mv, tag="xc")
                    nc.sync.dma_start(
                        xc[:],
                        xt_d[half * NKH * HD:(half + 1) * NKH * HD, ssl]
                        .rearrange("(k p) s -> p k s", p=HD))
                    xcs.append(xc)
                cs_sb = pt.tile([HD, SC], bf16, tag="cs")
                sn_sb = pt.tile([HD, SC], bf16, tag="sn")
                nc.sync.dma_start(cs_sb[:], cs_d[:, ssl])
                nc.sync.dma_start(sn_sb[:], sn_d[:, ssl])

                def xck(kt):
                    return xcs[kt // NKH][:, kt % NKH, :]

                yield

                def rope_block(raw_ps, out_ap):
                    raw = pr.tile([HD, SC], bf16, tag="rope_raw")
                    nc.vector.tensor_copy(out=raw[:], in_=raw_ps[:])
                    rot_ps = ps_r.tile([HD, SC], f32, tag="rot")
                    nc.tensor.matmul(rot_ps[:], rt_sb[:], raw[:],
                                     start=True, stop=True)
                    tmp = pr.tile([HD, SC], bf16, tag="rope_tmp")
                    nc.vector.tensor_mul(tmp[:], raw[:], cs_sb[:])
                    e1 = pr.tile([HD, SC], bf16, tag="rope_e1")
                    nc.vector.tensor_mul(e1[:], rot_ps[:], sn_sb[:])
                    nc.vector.tensor_add(out_ap, tmp[:], e1[:])

                for ft in range(HPC):
                    fsl = slice(ft * HD, (ft + 1) * HD)
                    q_ps = ps_q.tile([HD, SC], f32, tag="qk")
                    for kt in range(NKT):
                        nc.tensor.matmul(q_ps[:], wq_sb[:, kt, fsl], xck(kt),
                                         start=(kt == 0), stop=(kt == NKT - 1))
                    qro = pq.tile([HD, SC], bf16, tag="qro")
                    rope_block(q_ps, qro[:])
                    nc.sync.dma_start(qt_ds[sc][fsl, :], qro[:])
                    yield
                for ft in range(KVPC):
                    fsl = slice(ft * HD, (ft + 1) * HD)
                    k_ps = ps_q.tile([HD, SC], f32, tag="qk")
                    for kt in range(NKT):
                        nc.tensor.matmul(k_ps[:], wk_sb[:, kt, fsl], xck(kt),
                                         start=(kt == 0), stop=(kt == NKT - 1))
                    rope_block(k_ps, kt_sb[:, ft, ssl])
                    yield
                for st in range(SC // HD):
                    tsl = slice(st * HD, (st + 1) * HD)
                    v_ps = ps_v.tile([HD, FKV], f32, tag="v")
                    for kt in range(NKT):
                        nc.tensor.matmul(v_ps[:], xck(kt)[:, tsl], wv_sb[:, kt, :],
                                         start=(kt == 0), stop=(kt == NKT - 1))
                    nc.scalar.copy(
                        out=v_sb[:, sc * (SC // HD) + st, :], in_=v_ps[:])
                    yield

            # ---- phase B generator: one (qc, h) head at pair granularity ----
            def phase_b(qc, h, o_sb):
                kv = h // (HPC // KVPC)
                qt = pq.tile([HD, SC], bf16, tag="qt")
                nc.sync.dma_start(qt[:], qt_ds[qc][h * HD:(h + 1) * HD, :])
                ot_ps = ps_o.tile([HD, SC], f32, tag="ot")
                bc_ps = ps_b.tile([HD, SC], f32, tag="bc")
                nkt = 4 * qc + 4
                for kp in range(nkt // 2):
                    kt0 = 2 * kp
                    s_ps = ps_s.tile([HD, 2, SC], f32, tag="s")
                    e = pe.tile([HD, 2, SC], mv, tag="e")
                    if kt0 + 1 < 4 * qc:
                        for i in range(2):
                            kt = kt0 + i
                            nc.tensor.matmul(
                                s_ps[:, i, :],
                                kt_sb[:, kv, kt * HD:(kt + 1) * HD],
                                qt[:], start=True, stop=True)
                        nc.scalar.activation(e[:], s_ps[:], Exp, scale=SCALE)
                        for i in range(2):
                            kt = kt0 + i
                            nc.tensor.matmul(
                                ot_ps[:],
                                v_sb[:, kt, kv * HD:(kv + 1) * HD],
                                e[:, i, :], start=(kt == 0),
                                stop=(kt == nkt - 1), skip_group_check=True)
                            nc.tensor.matmul(
                                bc_ps[:], on_sb[:], e[:, i, :],
                                start=(kt == 0), stop=(kt == nkt - 1),
                                skip_group_check=True)
                    else:
                        # diagonal pair; r=3 widened to 256 cols (f32r
                        # <256-col penalty) and fixed up by mask mk3.
                        for i in range(2):
                            r = kt0 + i - 4 * qc
                            kt = kt0 + i
                            q0 = r * HD if r < 3 else 2 * HD
                            nc.tensor.matmul(
                                s_ps[:, i, q0:],
                                kt_sb[:, kv, kt * HD:(kt + 1) * HD],
                                qt[:, q0:], start=True, stop=True)
                            nc.scalar.activation(
                                e[:, i, q0:], s_ps[:, i, q0:], Exp,
                                scale=SCALE)
                            if r < 3:
                                nc.vector.tensor_mul(
                                    e[:, i, q0:q0 + HD],
                                    e[:, i, q0:q0 + HD],
                                    mk_sb[:, 0, :])
                            else:
                                nc.vector.tensor_mul(
                                    e[:, i, q0:],
                                    e[:, i, q0:],
                                    mk3_sb[:])
                            nc.tensor.matmul(
                                ot_ps[:, q0:],
                                v_sb[:, kt, kv * HD:(kv + 1) * HD],
                                e[:, i, q0:], start=(kt == 0),
                                stop=(kt == nkt - 1), skip_group_check=True)
                            nc.tensor.matmul(
                                bc_ps[:, q0:], on_sb[:], e[:, i, q0:],
                                start=(kt == 0), stop=(kt == nkt - 1),
                                skip_group_check=True)
                    yield
                inv = pn.tile([HD, SC], f32, tag="inv")
                nc.vector.reciprocal(inv[:], bc_ps[:])
                nc.vector.tensor_mul(o_sb[:, h, :], ot_ps[:], inv[:])
                yield

            # ---- phase C generator: one (qc, dt) output tile ----
            def phase_c(qc, o_sb):
                for dt in range(DIM // HD):
                    dsl = slice(dt * HD, (dt + 1) * HD)
                    y_ps = ps_y.tile([HD, SC], f32, tag="y")
                    for ft in range(HPC):
                        nc.tensor.matmul(
                            y_ps[:], wo_sb[:, ft, dsl], o_sb[:, ft, :],
                            start=(ft == 0), stop=(ft == HPC - 1))
                    y_sb = py.tile([HD, SC], bf16, tag="y_sb")
                    if dt % 2 == 0:
                        nc.scalar.copy(out=y_sb[:], in_=y_ps[:])
                    else:
                        nc.vector.tensor_copy(out=y_sb[:], in_=y_ps[:])
                    nc.sync.dma_start(
                        y_d[dsl, qc * SC:(qc + 1) * SC], y_sb[:])
                    yield

            # ---- interleaved schedule ----
            # Pull phase generators lazily so phase B's Activation-bound exp
            # stream interleaves with phase A/C PE-bound matmuls in the
            # per-engine instruction queues.
            o_tiles = {}

            for _ in phase_a(0):
                pass

            for qc in range(NSC):
                o_sb = po.tile([HD, HPC, SC], mv, tag="o_sb")
                o_tiles[qc] = o_sb
                bgens = [phase_b(qc, h, o_sb) for h in range(HPC)]
                fgens = []
                if qc + 1 < NSC:
                    fgens.append(phase_a(qc + 1))
                if qc - 1 >= 0:
                    fgens.append(phase_c(qc - 1, o_tiles[qc - 1]))
                # prefetch pull: issue next chunk's DMAs with a full B(qc)
                # of lead time
                f_steps = [g for g in fgens if _pull(g)]
                b_total = HPC * (2 * qc + 3)
                f_total = 14 * (1 if qc + 1 < NSC else 0) +                     16 * (1 if qc - 1 >= 0 else 0)
                done_f, step = 0, 0
                for h in range(HPC):
                    for _ in bgens[h]:
                        step += 1
                        want = (step * f_total) // b_total
                        while done_f < want and f_steps:
                            g = f_steps[done_f % len(f_steps)]
                            if _pull(g):
                                done_f += 1
                            else:
                                f_steps.remove(g)
                for g in f_steps:
                    for _ in g:
                        pass
            for _ in phase_c(NSC - 1, o_tiles[NSC - 1]):
                pass

    nc.finalize()
    return nc


def _rope_perm(nheads):
    idx = []
    for h in range(nheads):
        base = h * HD
        idx.extend(base + 2 * j for j in range(HD // 2))
        idx.extend(base + 2 * j + 1 for j in range(HD // 2))
    return np.array(idx)


def _prepare_in_maps(inputs):
    bf = ml_dtypes.bfloat16
    mvnp = np.float32 if MIXED else bf
    x = np.asarray(inputs["x"], dtype=np.float32)
    fc = np.asarray(inputs["freqs_cos"], dtype=np.float32)
    fs = np.asarray(inputs["freqs_sin"], dtype=np.float32)
    wq = np.asarray(inputs["wq"], dtype=np.float32)
    wk = np.asarray(inputs["wk"], dtype=np.float32)
    wv = np.asarray(inputs["wv"], dtype=np.float32)
    wo = np.asarray(inputs["wo"], dtype=np.float32)
    aq = np.asarray(inputs["aq"], dtype=np.float32)
    bq = np.asarray(inputs["bq"], dtype=np.float32)
    ak = np.asarray(inputs["ak"], dtype=np.float32)
    bk = np.asarray(inputs["bk"], dtype=np.float32)
    av = np.asarray(inputs["av"], dtype=np.float32)
    bv = np.asarray(inputs["bv"], dtype=np.float32)
    ao = np.asarray(inputs["ao"], dtype=np.float32)
    bo = np.asarray(inputs["bo"], dtype=np.float32)

    permQ = _rope_perm(HPC)
    permK = _rope_perm(KVPC)
    wq = (wq + bq.astype(np.float64) @ aq.astype(np.float64)).astype(np.float32)
    wk = (wk + bk.astype(np.float64) @ ak.astype(np.float64)).astype(np.float32)
    wv = (wv + bv.astype(np.float64) @ av.astype(np.float64)).astype(np.float32)
    wo = (wo + bo.astype(np.float64) @ ao.astype(np.float64)).astype(np.float32)
    fcT = np.ascontiguousarray(fc.T)
    fsT = np.ascontiguousarray(fs.T)
    cs = np.concatenate([fcT, fcT], axis=0).astype(bf)
    sn = np.concatenate([fsT, fsT], axis=0).astype(bf)
    rt = np.zeros((HD, HD), np.float32)
    for j in range(HD // 2):
        rt[j, 64 + j] = 1.0
        rt[64 + j, j] = -1.0
    rt = rt.astype(bf)
    ones = np.ones((HD, HD), np.float32).astype(mvnp)
    kk = np.arange(HD)[:, None]
    qq = np.arange(HD)[None, :]
    tri = (qq >= kk).astype(np.float32)
    mk = np.stack([tri, tri], axis=1)                   # [128, 2, 128]
    mk3 = np.concatenate([np.zeros((HD, HD), np.float32), tri], axis=1)

    xt_cache = {}
    in_maps = []
    for c in range(8):
        b, g = c // 2, c % 2
        if b not in xt_cache:
            xt_cache[b] = np.ascontiguousarray(x[b].T).astype(mvnp)
        fq = slice(g * FQ, (g + 1) * FQ)
        fkv = slice(g * FKV, (g + 1) * FKV)
        wq_g = wq[fq][permQ]
        wk_g = wk[fkv][permK]
        in_maps.append({
            "xt": xt_cache[b],
            "wq": np.ascontiguousarray(wq_g.T).astype(bf),
            "wk": np.ascontiguousarray(wk_g.T).astype(bf),
            "wv": np.ascontiguousarray(wv[fkv].T).astype(bf),
            "wo": np.ascontiguousarray(wo[:, fq].T).astype(bf),
            "cs": cs, "sn": sn, "rt": rt, "on": ones,
            "mk": mk, "mk3": mk3,
        })
    return in_maps


def _get_program():
    if "nc" not in _cache:
        _cache["nc"] = _build_program()
    return _cache["nc"]


def run(inputs, trace=False):
    from concourse import bass_utils
    nc = _get_program()
    in_maps = _prepare_in_maps(inputs)
    res = bass_utils.run_bass_kernel_spmd(
        nc, in_maps, list(range(8)), trace=trace)
    # y is [DIM, S] per core
    ys = [np.asarray(res.results[c]["y"]).astype(np.float32).T for c in range(8)]
    out = np.empty((B, S, DIM), np.float32)
    for b in range(B):
        out[b] = ys[2 * b] + ys[2 * b + 1]
    return out, res


def kernel(**inputs):
    out, _ = run(inputs, trace=False)
    return out


def bench(inputs, iters=20, n_cores=8):
    import time

    import jax
    import concourse.mybir as mybir
    from concourse import bass2jax
    from concourse.bass2jax import _bass_exec_p, partition_id_tensor
    from jax.sharding import Mesh, NamedSharding, PartitionSpec

    bass2jax.install_neuronx_cc_hook()
    nc = _get_program()
    in_maps = _prepare_in_maps(inputs)

    partition_name = nc.partition_id_tensor.name if nc.partition_id_tensor else None
    in_names, out_names, out_avals = [], [], []
    for alloc in nc.m.functions[0].allocations:
        if not isinstance(alloc, mybir.MemoryLocationSet):
            continue
        name = alloc.memorylocations[0].name
        if alloc.kind == "ExternalInput":
            if name != partition_name:
                in_names.append(name)
        elif alloc.kind == "ExternalOutput":
            out_names.append(name)
            out_avals.append(jax.core.ShapedArray(
                tuple(alloc.tensor_shape), mybir.dt.np(alloc.dtype)))
    n_params = len(in_names)
    all_names = list(in_names) + out_names
    if partition_name is not None:
        all_names.append(partition_name)

    def _body(*args):
        operands = list(args)
        if partition_name is not None:
            operands.append(partition_id_tensor())
        outs = _bass_exec_p.bind(
            *operands,
            out_avals=tuple(out_avals),
            in_names=tuple(all_names),
            out_names=tuple(out_names),
            lowering_input_output_aliases=(),
            sim_require_finite=True,
            sim_require_nnan=True,
            nc=nc,
        )
        return tuple(outs)

    devices = jax.devices()[:n_cores]
    mesh = Mesh(np.asarray(devices), ("core",))
    spec = NamedSharding(mesh, PartitionSpec("core"))
    from jax.experimental.shard_map import shard_map
    sharded = jax.jit(shard_map(
        _body, mesh=mesh,
        in_specs=(PartitionSpec("core"),) * (n_params + len(out_names)),
        out_specs=(PartitionSpec("core"),) * len(out_names),
        check_rep=False), keep_unused=True)

    concat_in = [
        jax.device_put(
            np.concatenate([np.asarray(in_maps[c][nm]) for c in range(n_cores)],
                           axis=0), spec)
        for nm in in_names]
    concat_zeros = [
        jax.device_put(
            np.zeros((n_cores * a.shape[0], *a.shape[1:]), a.dtype), spec)
        for a in out_avals]
    out = sharded(*concat_in, *concat_zeros)
    jax.block_until_ready(out)
    t0 = time.perf_counter()
    for _ in range(iters):
        out = sharded(*concat_in, *concat_zeros)
    jax.block_until_ready(out)
    t1 = time.perf_counter()

    if n_cores != 8:
        return (t1 - t0) / iters, None
    ys = np.asarray(out[out_names.index("y")]).reshape(n_cores, DIM, S)
    ys = ys.astype(np.float32)
    full = np.empty((B, S, DIM), np.float32)
    for b in range(B):
        full[b] = ys[2 * b].T + ys[2 * b + 1].T
    return (t1 - t0) / iters, full
